# revision 30
# baseline (speedup 1.0000x reference)
"""AMICO ADMM solver on 8 Trainium2 NeuronCores.

Problem: X = argmin ||Y^T - A x||^2 + lam*||x||_1 s.t. x >= 0, solved with
max_iter ADMM steps (rho=1, lam=0.1) exactly as in the reference scan.

Algebraic reduction (tracking only v = x + u):
    v_1 = G                      with G  = Minv @ A^T @ Y^T
    for i = 2..N:
        w   = |v - t|            (t = lam/rho)
        S   = min(v, t) + Gb     (Gb = G - t * Minv @ 1)
        v'  = Minv @ w + S
    output x_N = Minv @ w_{N-1} + Gb

since z = relu(v - t), u' = v - z = min(v, t), and z - u' = |v - t| - t.
The constant -t*Minv@1 and the A^T Y^T term are folded into a single
"augmented" matmul: Gb = Ht_aug^T @ Yt_aug where Ht_aug carries A@Minv plus a
bias row (-t * rowsum(Minv)) and Yt_aug carries Y^T plus a row of ones.

Sharding: data-parallel over voxels (B=4096 -> 512 per core); A-derived
matrices (Minv, Ht_aug) replicated; no cross-core communication.

Implementation notes (measured on silicon):
 - All matmul operands are fp16 (11-bit mantissa; fp32/fp16 mixing is
   rejected by the compiler, bf16 weights lose too much accuracy).
   End-to-end error vs the float32 cho_solve reference: ~5.7e-3.
   HW exec time (neuron-profile, whole NEFF): ~140 us in the chip's fast
   clock state, ~167 us when the chip self-derates (~18% clock variance
   between runs; per-iteration work is at the engine floor either way).
 - Output chunks 0,1 use a DVE V-op (v = psum + S) with fp16 v/S state so
   the S op hits the DVE 16-bit 2x mode; chunks 2,3 instead accumulate
   I @ S_comb into the PSUM group via an identity matmul, so v materializes
   directly in PSUM and the Abs activation reads it from there, shortening
   the cross-iteration chain and balancing PE vs DVE work.
"""

import numpy as np

B_VOX = 4096
M_MEAS = 256
K_ATOMS = 512
P = 128
N_CORES = 8
BS = B_VOX // N_CORES  # 512 voxels per core
KB = K_ATOMS // P  # 4 chunks of the contraction/output dim
LAM = 0.1
RHO = 1.0
THR = LAM / RHO

_NC_CACHE = {}


def _build(niter):
    import concourse.mybir as mybir
    import concourse.tile as tile
    from concourse import bacc

    f32 = mybir.dt.float32
    f16 = mybir.dt.float16
    Alu = mybir.AluOpType
    Act = mybir.ActivationFunctionType

    nc = bacc.Bacc(None, target_bir_lowering=False)
    # one host-pre-transposed packed param: per partition p the row holds
    # [Ht_kb0|Yt_kb0|Ht_kb1|Yt_kb1|Ht_kb2|Yt_kb2 | Id | rs | Mi] in fp16,
    # so every DMA descriptor is a multi-KB contiguous run and each G-round's
    # operands arrive in a single transfer.
    NHY = 2 * (K_ATOMS + BS)
    NMI = P + KB + KB * K_ATOMS  # id + cneg + mi in one tile
    NPACK = NHY + NMI
    packed = nc.declare_dram_parameter("packed", [P, NPACK], f16, isOutput=False)
    out = nc.declare_dram_parameter("out", [K_ATOMS, BS], f16, isOutput=True)
    MI0 = NHY
    MIW = P + KB  # weight columns start here inside mi_sb

    with tile.TileContext(nc) as tc:
        with (
            tc.tile_pool(name="const", bufs=1) as cpool,
            tc.tile_pool(name="v", bufs=4) as vpool,
            tc.tile_pool(name="w", bufs=9) as wpool,
            tc.tile_pool(name="s", bufs=3) as spool,
            tc.tile_pool(name="o", bufs=4) as opool,
            tc.tile_pool(name="psum", bufs=8, space="PSUM") as ppool,
        ):
            nb = cpool.tile([P, 1], f32)
            nc.vector.memset(nb[:], -THR)
            # parallel large-descriptor loads from the packed param
            hy_sb = cpool.tile([P, NHY], f16)
            _kbw = K_ATOMS + BS
            mi_sb = cpool.tile([P, NMI], f16)
            # split at weight-chunk boundaries: [id+cneg+kb0 | kb1 | kb2 | kb3]
            _cuts = [0, MIW + K_ATOMS, MIW + 2 * K_ATOMS, MIW + 3 * K_ATOMS, NMI]
            nc.sync.dma_start(hy_sb[:, 0:_kbw], packed[:, 0:_kbw])
            nc.sync.dma_start(
                mi_sb[:, 0 : _cuts[1]], packed[:, MI0 : MI0 + _cuts[1]]
            )
            nc.sync.dma_start(hy_sb[:, _kbw:], packed[:, _kbw:NHY])
            for _c in range(1, 4):
                nc.sync.dma_start(
                    mi_sb[:, _cuts[_c] : _cuts[_c + 1]],
                    packed[:, MI0 + _cuts[_c] : MI0 + _cuts[_c + 1]],
                )
            id_sb = mi_sb[:, 0:P]
            cn_sb = cpool.tile([P, KB], f32)
            nc.vector.tensor_copy(cn_sb[:], mi_sb[:, P : P + KB])
            gb16_sb = cpool.tile([P, KB, BS], f16)  # Gb (fp16) for S-ops and final x

            outr = out.rearrange("(mb p) n -> p mb n", p=P)

            w_cur = [None] * KB
            s_cur = [None] * KB  # chunks 0,1: S (f16); chunks 2,3: S_comb (f16)

            # ---- iteration 1: G = H^T @ Yt (m-outer blocks); v_1 = G stays
            # in PSUM and Gb = G - t*rowsum(Minv) is applied as a per-
            # partition ACT bias during the copy to SBUF. ----
            pgs = [
                ppool.tile([P, BS], f32, tag="pp", name=f"pg{m}") for m in range(KB)
            ]
            for kb in range(2):
                for m in range(KB):
                    nc.tensor.matmul(
                        pgs[m][:],
                        lhsT=hy_sb[:, kb * _kbw + m * P : kb * _kbw + (m + 1) * P],
                        rhs=hy_sb[:, kb * _kbw + K_ATOMS : (kb + 1) * _kbw],
                        start=(kb == 0),
                        stop=(kb == 1),
                    )
            for m in range(KB):
                if niter == 1:
                    xm = opool.tile([P, BS], f16, tag="x", name=f"x1{m}")
                    nc.vector.tensor_copy(xm[:], pgs[m][:])
                    nc.sync.dma_start(outr[:, m, :], xm[:])
                else:
                    wm = wpool.tile([P, BS], f16, tag="w", name=f"w1{m}")
                    nc.scalar.activation(wm[:], pgs[m][:], Act.Abs, bias=nb[:, 0:1])
                    # Gb to SBUF with the -t*rowsum bias folded in
                    nc.scalar.activation(
                        gb16_sb[:, m, :], pgs[m][:], Act.Identity,
                        bias=cn_sb[:, m : m + 1],
                    )
                    sm = spool.tile([P, BS], f16, tag=f"s{m}", name=f"s1{m}")
                    nc.vector.scalar_tensor_tensor(
                        sm[:], pgs[m][:], THR, gb16_sb[:, m, :], Alu.min, Alu.add
                    )
                    w_cur[m], s_cur[m] = wm, sm

            # ---- iterations 2..niter ----
            for it in range(2, niter + 1):
                last = it == niter
                pps = [
                    ppool.tile([P, BS], f32, tag="pp", name=f"pp{it}_{m}")
                    for m in range(KB)
                ]
                vs = [None, None]
                neww = [None] * KB
                news = [None] * KB
                for m in range(KB):
                    use_ident = (m >= 2) or last
                    if use_ident:
                        # accumulate directly in PSUM: I @ S_comb (+Gb on the
                        # last iteration) + Minv @ w
                        nc.tensor.matmul(
                            pps[m][:],
                            lhsT=id_sb[:],
                            rhs=gb16_sb[:, m, :] if last else s_cur[m][:],
                            start=True,
                            stop=False,
                        )
                    for kb in range(KB):
                        nc.tensor.matmul(
                            pps[m][:],
                            lhsT=mi_sb[:, MIW + kb * K_ATOMS + m * P : MIW + kb * K_ATOMS + (m + 1) * P],
                            rhs=w_cur[kb][:],
                            start=(kb == 0) and not use_ident,
                            stop=(kb == KB - 1),
                        )
                for m in range(KB):
                    if last:
                        xm = opool.tile([P, BS], f16, tag="x", name=f"x{m}")
                        nc.scalar.activation(xm[:], pps[m][:], Act.Copy)
                        nc.sync.dma_start(outr[:, m, :], xm[:])
                    elif m < 2:
                        # V-op: v = psum + S_prev (critical chain)
                        vm = vpool.tile([P, BS], f16, tag="v", name=f"v{it}_{m}")
                        nc.vector.scalar_tensor_tensor(
                            vm[:], pps[m][:], 0.0, s_cur[m][:], Alu.bypass, Alu.add
                        )
                        vs[m] = vm
                        wm = wpool.tile([P, BS], f16, tag="w", name=f"w{it}_{m}")
                        nc.scalar.activation(wm[:], vm[:], Act.Abs, bias=nb[:, 0:1])
                        neww[m] = wm
                    else:
                        # v lives in PSUM; ACT reads it directly
                        wm = wpool.tile([P, BS], f16, tag="w", name=f"w{it}_{m}")
                        nc.scalar.activation(wm[:], pps[m][:], Act.Abs, bias=nb[:, 0:1])
                        neww[m] = wm
                        sm = spool.tile([P, BS], f16, tag=f"s{m}", name=f"s{it}_{m}")
                        nc.vector.scalar_tensor_tensor(
                            sm[:], pps[m][:], THR, gb16_sb[:, m, :], Alu.min, Alu.add
                        )
                        news[m] = sm
                if not last:
                    # S ops for chunks 0,1 (off the critical chain)
                    for m in range(2):
                        sm = spool.tile([P, BS], f16, tag=f"s{m}", name=f"s{it}_{m}")
                        nc.vector.scalar_tensor_tensor(
                            sm[:], vs[m][:], THR, gb16_sb[:, m, :], Alu.min, Alu.add
                        )
                        news[m] = sm
                    w_cur, s_cur = neww, news

    nc.finalize()
    return nc


def _get_nc(niter):
    if niter not in _NC_CACHE:
        _NC_CACHE[niter] = _build(niter)
    return _NC_CACHE[niter]


def _prep_in_maps(Y, A):
    """Host precompute of the A-derived (voxel-independent) factor matrices,
    in float64: the inverse replaces the reference's Cholesky solve. Shards Y
    over voxels (transposed + augmented ones-row) and packs all device inputs
    into one pre-transposed [128, NPACK] fp16 array so every DMA descriptor
    is a multi-KB contiguous run."""
    A64 = A.astype(np.float64)
    LHS = A64.T @ A64 + RHO * np.eye(K_ATOMS)
    Minv = np.linalg.inv(LHS)
    Minv = (Minv + Minv.T) / 2
    Hm = A64 @ Minv  # [M, K]
    rsum = Minv.sum(axis=1)

    Ht = Hm.astype(np.float16)  # [M, K], M = 2*P exactly
    htp = Ht.reshape(2, P, K_ATOMS).transpose(1, 0, 2)  # [P, 2, K]
    Mi = Minv.astype(np.float16)
    mip = Mi.reshape(KB, P, K_ATOMS).transpose(1, 0, 2).reshape(P, KB * K_ATOMS)
    cneg = (-THR * rsum).astype(np.float16).reshape(KB, P).T  # [P, KB]
    Id = np.eye(P, dtype=np.float16)
    fixed = np.concatenate([Id, cneg, mip], axis=1)  # [P, P + KB + KB*K]

    in_maps = []
    for c in range(N_CORES):
        Yt = Y[c * BS : (c + 1) * BS, :].T.astype(np.float16)  # [M, BS]
        ytp = Yt.reshape(2, P, BS).transpose(1, 0, 2)  # [P, 2, BS]
        hy = np.concatenate([htp, ytp], axis=2).reshape(P, 2 * (K_ATOMS + BS))
        packed = np.ascontiguousarray(np.concatenate([hy, fixed], axis=1))
        in_maps.append({"packed": packed})
    return in_maps


def kernel(Y, A, max_iter):
    from concourse.bass_utils import run_bass_kernel_spmd

    Y = np.ascontiguousarray(np.asarray(Y, dtype=np.float32))
    A = np.ascontiguousarray(np.asarray(A, dtype=np.float32))
    niter = int(max_iter)
    assert Y.shape == (B_VOX, M_MEAS) and A.shape == (M_MEAS, K_ATOMS)
    if niter < 1:
        # zero-length scan returns the zero initial state
        return np.zeros((B_VOX, K_ATOMS), np.float32)

    in_maps = _prep_in_maps(Y, A)
    nc = _get_nc(niter)
    res = run_bass_kernel_spmd(nc, in_maps, core_ids=list(range(N_CORES)))

    outp = np.empty((B_VOX, K_ATOMS), np.float32)
    for c in range(N_CORES):
        outp[c * BS : (c + 1) * BS] = res.results[c]["out"].T.astype(np.float32)
    return outp
